# revision 10
# baseline (speedup 1.0000x reference)
"""AttnBlock6 Trainium2 kernel (Bass/Tile, 8 NeuronCores).

GroupNorm -> qkv 1x1conv -> patch-local attention + pooled global attention
-> combine -> proj -> residual.  x: [2, 64, 448, 448] fp32.

Sharding: sample b -> cores 4b..4b+3; each core owns 112 image rows
(quarter sample, 64ch x 50176px).  Per-sample collectives (stats, wm,
pooled kg/vg) over 4-core replica groups.
"""
import numpy as np

C = 64
SIZE = 448
P2 = 196
TG = 56
POOL = 8
EPS = 1e-5
NCORES = 8
QROWS = 112                   # image rows per core
Q = QROWS * SIZE              # 50176 pixels per core
HALF_T = 7                    # pass-B tiles per ring half
TB = 3584                     # pass-B tile: 8 image rows
TC = 3136                     # pass-C tile: 16 chunks = 7 image rows
PROWS = QROWS // POOL         # 14 pooled rows per core
PQ = PROWS * TG               # 784 pooled pixels per core
GP = 4 * PQ                   # 3136 pooled pixels per sample
NSAMP_EL = C * 4 * Q          # elements per sample (for mean)

_cache = {}


def _build():
    import concourse.bass as bass
    import concourse.tile as tile
    import concourse.mybir as mybir
    from concourse import bacc
    from concourse.masks import make_identity

    dt = mybir.dt
    f32, f32r, bf16 = dt.float32, dt.float32r, dt.bfloat16
    AF = mybir.ActivationFunctionType
    ALU = mybir.AluOpType
    AX = mybir.AxisListType
    PF = mybir.PoolFunctionType
    ts = bass.ts

    nc = bacc.Bacc("TRN2", target_bir_lowering=False, debug=False,
                   num_devices=NCORES)

    xq = nc.dram_tensor("xq", [C, Q], bf16, kind="ExternalInput")
    wq_t = nc.dram_tensor("wq_t", [C, C], f32, kind="ExternalInput")
    wk_t = nc.dram_tensor("wk_t", [C, C], f32, kind="ExternalInput")
    wv_ext = nc.dram_tensor("wv_ext", [C + 1, C], f32, kind="ExternalInput")
    pw_t = nc.dram_tensor("pw_t", [C, C], f32, kind="ExternalInput")
    bqk = nc.dram_tensor("bqk", [C, 3], f32, kind="ExternalInput")
    outq = nc.dram_tensor("outq", [C, Q], bf16, kind="ExternalOutput")

    st_in = nc.dram_tensor("st_in", [1, 2], f32, kind="Internal")
    st_out = nc.dram_tensor("st_out", [1, 2], f32, kind="Internal")
    kv_in = nc.dram_tensor("kv_in", [2, C, PQ], bf16, kind="Internal")
    kv_out = nc.dram_tensor("kv_out", [4, 2, C, PQ], bf16, kind="Internal")
    wm_in = nc.dram_tensor("wm_in", [98, 392], f32, kind="Internal")
    wm_out = nc.dram_tensor("wm_out", [98, 392], f32, kind="Internal")

    GROUPS = [[0, 1, 2, 3], [4, 5, 6, 7]]

    with tile.TileContext(nc) as tc:
        with tc.tile_pool(name="persist", bufs=1) as persist:
            # ---- persistent small tensors ----
            w_q = persist.tile([C, C], f32)
            w_k = persist.tile([C, C], f32)
            w_v = persist.tile([C + 1, C], f32)
            w_p = persist.tile([C, C], f32)
            b2 = persist.tile([C, 3], f32)
            nc.sync.dma_start(w_q[:], wq_t[:])
            nc.sync.dma_start(w_k[:], wk_t[:])
            nc.sync.dma_start(w_v[:], wv_ext[:])
            nc.sync.dma_start(w_p[:], pw_t[:])
            nc.sync.dma_start(b2[:], bqk[:])
            w_q_bf = persist.tile([C, C], bf16)
            w_k_bf = persist.tile([C, C], bf16)
            w_v_bf = persist.tile([C + 1, C], bf16)
            nc.scalar.copy(w_q_bf[:], w_q[:])
            nc.scalar.copy(w_k_bf[:], w_k[:])
            nc.scalar.copy(w_v_bf[:], w_v[:])
            w_p_bf = persist.tile([C, C], bf16)
            nc.scalar.copy(w_p_bf[:], w_p[:])
            id_bf = persist.tile([128, 128], bf16)
            make_identity(nc, id_bf[:])
            ones_bf = persist.tile([128, 1], bf16)
            nc.gpsimd.memset(ones_bf[:], 1.0)
            ones_f = persist.tile([128, 1], f32)
            nc.gpsimd.memset(ones_f[:], 1.0)
            inv64 = persist.tile([C, 1], f32)
            nb64 = persist.tile([C, 1], f32)
            qg = persist.tile([C, PQ], bf16)
            kg_full = persist.tile([C, GP], bf16)
            vg_full = persist.tile([C, GP], bf16)
            wmT0 = persist.tile([128, P2], bf16)
            wmT1 = persist.tile([68, P2], bf16)
            hgw = persist.tile([C, PROWS * SIZE], f32)
            px_own = persist.tile([C, PQ], f32)
            wm_sb = persist.tile([98, 392], f32)

            # =========== pass A: stats ===========
            with tc.tile_pool(name="pa", bufs=2) as pa, \
                 tc.tile_pool(name="pa_ps", bufs=1, space="PSUM") as pa_ps:
                xv = xq[:].rearrange("c (g f) -> (c g) f", g=2)  # [128, 25088]
                scol = pa.tile([128, 8], f32, tag="scol")
                qcol = pa.tile([128, 8], f32, tag="qcol")
                for t in range(8):
                    st = pa.tile([128, 3136], bf16, tag="stat_in")
                    nc.sync.dma_start(st[:], xv[:, ts(t, 3136)])
                    sq = pa.tile([128, 3136], f32, tag="stat_sq")
                    nc.scalar.activation(sq[:], st[:], AF.Square,
                                         accum_out=qcol[:, t:t + 1])
                    nc.vector.tensor_reduce(scol[:, t:t + 1], st[:],
                                            axis=AX.X, op=ALU.add)
                s2 = pa.tile([128, 2], f32, tag="s2")
                nc.vector.tensor_reduce(s2[:, 0:1], scol[:], axis=AX.X, op=ALU.add)
                nc.vector.tensor_reduce(s2[:, 1:2], qcol[:], axis=AX.X, op=ALU.add)
                ps_st = pa_ps.tile([1, 2], f32, tag="ps_st")
                nc.tensor.matmul(ps_st[:], ones_f[:], s2[:], start=True, stop=True)
                st_sb = pa.tile([1, 2], f32, tag="st_sb")
                nc.vector.tensor_copy(st_sb[:], ps_st[:])
                nc.sync.dma_start(st_in[:], st_sb[:])
                nc.gpsimd.collective_compute(
                    "AllReduce", ALU.add, replica_groups=GROUPS,
                    ins=[st_in[:].opt()], outs=[st_out[:].opt()])
                st_g = pa.tile([1, 2], f32, tag="st_g")
                nc.gpsimd.dma_start(st_g[:], st_out[:])

                mean = pa.tile([1, 1], f32, tag="mean")
                ex2 = pa.tile([1, 1], f32, tag="ex2")
                nc.scalar.mul(mean[:], st_g[:, 0:1], 1.0 / NSAMP_EL)
                nc.scalar.mul(ex2[:], st_g[:, 1:2], 1.0 / NSAMP_EL)
                m2 = pa.tile([1, 1], f32, tag="m2")
                nc.vector.tensor_mul(m2[:], mean[:], mean[:])
                var = pa.tile([1, 1], f32, tag="var")
                nc.vector.tensor_sub(var[:], ex2[:], m2[:])
                sd = pa.tile([1, 1], f32, tag="sd")
                eps_t = pa.tile([1, 1], f32, tag="eps")
                nc.gpsimd.memset(eps_t[:], EPS)
                nc.scalar.activation(sd[:], var[:], AF.Sqrt, bias=eps_t[:])
                inv = pa.tile([1, 1], f32, tag="inv")
                nc.vector.reciprocal(inv[:], sd[:])
                nb0 = pa.tile([1, 1], f32, tag="nb0")
                nc.vector.tensor_mul(nb0[:], mean[:], inv[:])
                nb1 = pa.tile([1, 1], f32, tag="nb1")
                nc.scalar.mul(nb1[:], nb0[:], -1.0)
                nc.gpsimd.partition_broadcast(inv64[:], inv[:], channels=C)
                nc.gpsimd.partition_broadcast(nb64[:], nb1[:], channels=C)

            # =========== pass B: q,k convs + pooling + wm ===========
            with tc.tile_pool(name="pb", bufs=2) as pb, \
                 tc.tile_pool(name="pb_ring", bufs=1) as pb_ring, \
                 tc.tile_pool(name="pb_ps", bufs=2, space="PSUM") as pb_ps, \
                 tc.tile_pool(name="pb_psa", bufs=1, space="PSUM") as pb_psa:
                q_ring = pb_ring.tile([C, HALF_T * TB], bf16)
                k_ring = pb_ring.tile([C, HALF_T * TB], bf16)
                ps_w0 = pb_psa.tile([98, P2], f32, tag="wm0")
                ps_w1 = pb_psa.tile([98, P2], f32, tag="wm1")

                for half in range(2):
                    for tt in range(HALF_T):
                        t = half * HALF_T + tt
                        x_t = pb.tile([C, TB], bf16, tag="bx")
                        nc.sync.dma_start(x_t[:], xq[:, ts(t, TB)])
                        xn_t = pb.tile([C, TB], bf16, tag="bxn")
                        nc.scalar.activation(xn_t[:], x_t[:], AF.Identity,
                                             bias=nb64[:], scale=inv64[:])
                        for j in range(7):
                            ps_q = pb_ps.tile([C, 512], f32, tag="ps_q")
                            nc.tensor.matmul(ps_q[:], w_q_bf[:],
                                             xn_t[:, ts(j, 512)],
                                             start=True, stop=True)
                            nc.vector.tensor_scalar(
                                q_ring[:, tt * TB + j * 512:
                                       tt * TB + (j + 1) * 512],
                                ps_q[:], b2[:, 0:1], None, op0=ALU.add)
                            ps_k = pb_ps.tile([C, 512], f32, tag="ps_k")
                            nc.tensor.matmul(ps_k[:], w_k_bf[:],
                                             xn_t[:, ts(j, 512)],
                                             start=True, stop=True)
                            nc.scalar.activation(
                                k_ring[:, tt * TB + j * 512:
                                       tt * TB + (j + 1) * 512],
                                ps_k[:], AF.Identity, bias=b2[:, 1:2])
                        # pool raw x: 8x8 avg
                        px1 = pb.tile([C, 448], f32, tag="px1")
                        nc.vector.tensor_reduce(
                            px1[:],
                            x_t[:].rearrange("p (r g e) -> p r g e",
                                             r=8, g=TG, e=POOL),
                            axis=AX.X, op=ALU.add)
                        nc.vector.tensor_reduce(
                            px_own[:, ts(t, TG)],
                            px1[:].rearrange("p (r g) -> p g r", r=8, g=TG),
                            axis=AX.X, op=ALU.add)
                    # wm accumulation for this half's 128 chunks
                    for jc in range(128):
                        co = jc * P2
                        nc.tensor.matmul(ps_w0[:], q_ring[:, co: co + 98],
                                         k_ring[:, co: co + P2],
                                         start=(half == 0 and jc == 0),
                                         stop=(half == 1 and jc == 127))
                        nc.tensor.matmul(ps_w1[:], q_ring[:, co + 98: co + P2],
                                         k_ring[:, co: co + P2],
                                         start=(half == 0 and jc == 0),
                                         stop=(half == 1 and jc == 127))
                nc.vector.tensor_copy(wm_sb[:, 0:P2], ps_w0[:])
                nc.vector.tensor_copy(wm_sb[:, P2:392], ps_w1[:])

            # pooled convs + kv AllGather + wm AllReduce
            with tc.tile_pool(name="pg", bufs=2) as pg, \
                 tc.tile_pool(name="pg_ps", bufs=2, space="PSUM") as pg_ps:
                xg = pg.tile([C, PQ], bf16, tag="xg")
                inv64d = pg.tile([C, 1], f32, tag="inv64d")
                nc.scalar.mul(inv64d[:], inv64[:], 1.0 / 64.0)
                nc.scalar.activation(xg[:], px_own[:], AF.Identity,
                                     bias=nb64[:], scale=inv64d[:])
                kvg = pg.tile([C, 2 * PQ], bf16, tag="kvg")
                for h in range(2):
                    ps_g = pg_ps.tile([C, 392], f32, tag="ps_g")
                    nc.tensor.matmul(ps_g[:], w_q_bf[:],
                                     xg[:, ts(h, 392)],
                                     start=True, stop=True)
                    nc.vector.tensor_scalar(qg[:, ts(h, 392)], ps_g[:],
                                            b2[:, 0:1], None, op0=ALU.add)
                    ps_g2 = pg_ps.tile([C, 392], f32, tag="ps_g")
                    nc.tensor.matmul(ps_g2[:], w_k_bf[:],
                                     xg[:, ts(h, 392)],
                                     start=True, stop=True)
                    nc.vector.tensor_scalar(kvg[:, ts(h, 392)], ps_g2[:],
                                            b2[:, 1:2], None, op0=ALU.add)
                    ps_g3 = pg_ps.tile([C, 392], f32, tag="ps_g")
                    nc.tensor.matmul(ps_g3[:], w_v_bf[0:C, :],
                                     xg[:, ts(h, 392)],
                                     start=True, stop=True)
                    nc.vector.tensor_scalar(kvg[:, PQ + h * 392: PQ + (h + 1) * 392],
                                            ps_g3[:], b2[:, 2:3], None, op0=ALU.add)
                nc.sync.dma_start(kv_in[0], kvg[:, 0: PQ])
                nc.sync.dma_start(kv_in[1], kvg[:, PQ: 2 * PQ])
                nc.gpsimd.collective_compute(
                    "AllGather", ALU.bypass, replica_groups=GROUPS,
                    ins=[kv_in[:].opt()], outs=[kv_out[:].opt()])
                nc.sync.dma_start(
                    kg_full[:].rearrange("c (r f) -> c r f", r=4),
                    kv_out[:, 0].rearrange("r c f -> c r f"))
                nc.sync.dma_start(
                    vg_full[:].rearrange("c (r f) -> c r f", r=4),
                    kv_out[:, 1].rearrange("r c f -> c r f"))

                nc.sync.dma_start(wm_in[:], wm_sb[:])
                nc.gpsimd.collective_compute(
                    "AllReduce", ALU.add, replica_groups=GROUPS,
                    ins=[wm_in[:].opt()], outs=[wm_out[:].opt()])
                wm_g = pg.tile([98, 392], f32, tag="wm_g")
                nc.gpsimd.dma_start(wm_g[:], wm_out[:])

                # softmax rows (scale S^-0.5), fold 0.75, cast bf16
                wmn = pg.tile([98, 392], bf16, tag="wmn")
                SM = float(1.0 / np.sqrt(C * 1024.0))
                for h in range(2):
                    e_h = pg.tile([98, P2], bf16, tag="wm_e")
                    d_h = pg.tile([98, 1], f32, tag="wm_d")
                    nc.scalar.activation(e_h[:], wm_g[:, ts(h, P2)], AF.Exp,
                                         scale=SM, accum_out=d_h[:])
                    r_h = pg.tile([98, 1], f32, tag="wm_r")
                    nc.vector.reciprocal(r_h[:], d_h[:])
                    r_h2 = pg.tile([98, 1], f32, tag="wm_r2")
                    nc.scalar.mul(r_h2[:], r_h[:], 0.75)
                    nc.vector.tensor_scalar(wmn[:, ts(h, P2)], e_h[:],
                                            r_h2[:], None, op0=ALU.mult)
                # transpose wmn -> wmT0 [128,196], wmT1 [68,196]
                for h in range(2):
                    ps_t0 = pg_ps.tile([128, 98], bf16, tag="ps_t0")
                    nc.tensor.transpose(ps_t0[:], wmn[:, h * P2: h * P2 + 128],
                                        id_bf[0:98, 0:98])
                    nc.scalar.copy(wmT0[:, h * 98:(h + 1) * 98], ps_t0[:])
                    ps_t1 = pg_ps.tile([68, 98], bf16, tag="ps_t1")
                    nc.tensor.transpose(ps_t1[:],
                                        wmn[:, h * P2 + 128: (h + 1) * P2],
                                        id_bf[0:98, 0:98])
                    nc.scalar.copy(wmT1[:, h * 98:(h + 1) * 98], ps_t1[:])

            # =========== global attention ===========
            KT = [128] * 24 + [64]
            with tc.tile_pool(name="ga", bufs=1) as ga, \
                 tc.tile_pool(name="ga_ps", bufs=2, space="PSUM") as ga_ps:
                expT = ga.tile([128, 25 * PQ], bf16)
                vgT = ga.tile([128, 25 * C], bf16)
                for kt in range(25):
                    ksz = KT[kt]
                    ko = kt * 128
                    for h in range(2):
                        ps_wg = ga_ps.tile([128, 392], f32, tag="ps_wg")
                        nc.tensor.matmul(ps_wg[0:ksz, :],
                                         kg_full[:, ko: ko + ksz],
                                         qg[:, ts(h, 392)],
                                         start=True, stop=True)
                        nc.scalar.activation(
                            expT[0:ksz, kt * PQ + h * 392:
                                 kt * PQ + (h + 1) * 392],
                            ps_wg[0:ksz, :], AF.Exp, scale=0.125)
                    ps_vt = ga_ps.tile([128, C], bf16, tag="ps_vt")
                    nc.tensor.transpose(ps_vt[0:ksz, :], vg_full[:, ko: ko + ksz],
                                        id_bf[0:C, 0:C])
                    nc.scalar.copy(vgT[0:ksz, ts(kt, C)], ps_vt[0:ksz, :])
                with tc.tile_pool(name="ga2_ps", bufs=1, space="PSUM") as ga2_ps:
                    ps_d0 = ga2_ps.tile([1, 392], f32, tag="ps_d0")
                    ps_d1 = ga2_ps.tile([1, 392], f32, tag="ps_d1")
                    ps_h0 = ga2_ps.tile([C, 392], f32, tag="ps_h0")
                    ps_h1 = ga2_ps.tile([C, 392], f32, tag="ps_h1")
                    for kt in range(25):
                        ksz = KT[kt]
                        st_, sp_ = (kt == 0), (kt == 24)
                        e0 = expT[0:ksz, kt * PQ: kt * PQ + 392]
                        e1 = expT[0:ksz, kt * PQ + 392: (kt + 1) * PQ]
                        nc.tensor.matmul(ps_d0[:], ones_bf[0:ksz, :], e0,
                                         start=st_, stop=sp_)
                        nc.tensor.matmul(ps_d1[:], ones_bf[0:ksz, :], e1,
                                         start=st_, stop=sp_)
                        nc.tensor.matmul(ps_h0[:], vgT[0:ksz, ts(kt, C)], e0,
                                         start=st_, stop=sp_)
                        nc.tensor.matmul(ps_h1[:], vgT[0:ksz, ts(kt, C)], e1,
                                         start=st_, stop=sp_)
                    d_sb = ga.tile([1, PQ], f32)
                    nc.vector.tensor_copy(d_sb[:, 0:392], ps_d0[:])
                    nc.vector.tensor_copy(d_sb[:, 392:PQ], ps_d1[:])
                    rd = ga.tile([1, PQ], f32)
                    nc.vector.reciprocal(rd[:], d_sb[:])
                    rd2 = ga.tile([1, PQ], f32)
                    nc.scalar.mul(rd2[:], rd[:], 0.25)
                    rd64 = ga.tile([C, PQ], f32)
                    nc.gpsimd.partition_broadcast(rd64[:], rd2[:], channels=C)
                    hgn = ga.tile([C, PQ], f32)
                    nc.vector.tensor_mul(hgn[:, 0:392], ps_h0[:], rd64[:, 0:392])
                    nc.vector.tensor_mul(hgn[:, 392:PQ], ps_h1[:],
                                         rd64[:, 392:PQ])
                    hgw_v = hgw[:].rearrange("p (r g e) -> p r g e",
                                             r=PROWS, g=TG, e=POOL)
                    hgn_v = hgn[:].rearrange("p (r g e) -> p r g e",
                                             r=PROWS, g=TG, e=1)
                    for e in range(POOL):
                        nc.vector.tensor_copy(hgw_v[:, :, :, e:e + 1], hgn_v[:])

            # =========== pass C ===========
            with tc.tile_pool(name="pc", bufs=2) as pc, \
                 tc.tile_pool(name="pc_ps", bufs=2, space="PSUM") as pc_ps:
                for t in range(16):
                    x_t = pc.tile([C, TC], bf16, tag="cx")
                    nc.sync.dma_start(x_t[:], xq[:, ts(t, TC)])
                    xn_bf = pc.tile([C + 1, TC], bf16, tag="cxn")
                    nc.scalar.activation(xn_bf[0:C, :], x_t[:], AF.Identity,
                                         bias=nb64[:], scale=inv64[:])
                    nc.gpsimd.memset(xn_bf[C:C + 1, :], 1.0)
                    vTa = pc.tile([128, 16 * C], bf16, tag="vTa")
                    vTb = pc.tile([68, 16 * C], bf16, tag="vTb")
                    for u in range(16):
                        po = u * P2
                        ps_v0 = pc_ps.tile([128, C], f32, tag="ps_v0")
                        nc.tensor.matmul(ps_v0[:], xn_bf[:, po: po + 128],
                                         w_v_bf[:], start=True, stop=True)
                        nc.scalar.copy(vTa[:, ts(u, C)], ps_v0[:])
                        ps_v1 = pc_ps.tile([68, C], f32, tag="ps_v1")
                        nc.tensor.matmul(ps_v1[:], xn_bf[:, po + 128: po + P2],
                                         w_v_bf[:], start=True, stop=True)
                        nc.scalar.copy(vTb[:, ts(u, C)], ps_v1[:])
                    h_in = pc.tile([C, TC], bf16, tag="h_in")
                    for u in range(16):
                        ps_hp = pc_ps.tile([C, P2], f32, tag="ps_hp")
                        nc.tensor.matmul(ps_hp[:], vTa[:, ts(u, C)], wmT0[:],
                                         start=True, stop=False)
                        nc.tensor.matmul(ps_hp[:], vTb[:, ts(u, C)], wmT1[:],
                                         start=False, stop=True)
                        p0 = u * P2
                        while p0 < (u + 1) * P2:
                            r = p0 // SIZE
                            seg = min((u + 1) * P2, (r + 1) * SIZE) - p0
                            gr = (t * 7 + r) // POOL
                            w0 = p0 - r * SIZE
                            nc.vector.tensor_add(
                                h_in[:, p0: p0 + seg],
                                ps_hp[:, p0 - u * P2: p0 - u * P2 + seg],
                                hgw[:, gr * SIZE + w0: gr * SIZE + w0 + seg])
                            p0 += seg
                    out_t = pc.tile([C, TC], bf16, tag="cout")
                    for j in range(7):
                        ps_o = pc_ps.tile([C, SIZE], f32, tag="ps_o")
                        nc.tensor.matmul(ps_o[:], w_p_bf[:],
                                         h_in[:, ts(j, SIZE)],
                                         start=True, stop=True)
                        nc.vector.tensor_add(out_t[:, ts(j, SIZE)], ps_o[:],
                                             x_t[:, ts(j, SIZE)])
                    nc.sync.dma_start(outq[:, ts(t, TC)], out_t[:])

    nc.finalize()
    return nc


def _get_runner():
    if "runner" in _cache:
        return _cache["runner"]
    import jax
    import jax.numpy as jnp
    import concourse.mybir as mybir
    from jax.experimental.shard_map import shard_map
    from jax.sharding import Mesh, PartitionSpec, NamedSharding
    from concourse.bass2jax import (_bass_exec_p, partition_id_tensor,
                                    install_neuronx_cc_hook)

    nc = _build()
    install_neuronx_cc_hook()
    partition_name = (nc.partition_id_tensor.name
                      if nc.partition_id_tensor else None)
    in_names, out_names, out_avals = [], [], []
    for alloc in nc.m.functions[0].allocations:
        if not isinstance(alloc, mybir.MemoryLocationSet):
            continue
        name = alloc.memorylocations[0].name
        if alloc.kind == "ExternalInput":
            if name != partition_name:
                in_names.append(name)
        elif alloc.kind == "ExternalOutput":
            out_names.append(name)
            out_avals.append(jax.core.ShapedArray(
                tuple(alloc.tensor_shape), mybir.dt.np(alloc.dtype)))
    n_params = len(in_names)
    n_outs = len(out_avals)
    all_in_names = in_names + out_names
    if partition_name is not None:
        all_in_names = all_in_names + [partition_name]
    donate = tuple(range(n_params, n_params + n_outs))

    def _body(*args):
        operands = list(args)
        if partition_name is not None:
            operands.append(partition_id_tensor())
        outs = _bass_exec_p.bind(
            *operands, out_avals=tuple(out_avals),
            in_names=tuple(all_in_names), out_names=tuple(out_names),
            lowering_input_output_aliases=(), sim_require_finite=True,
            sim_require_nnan=True, nc=nc)
        return tuple(outs)

    devices = jax.devices()[:NCORES]
    mesh = Mesh(np.asarray(devices), ("core",))
    in_specs = (PartitionSpec("core"),) * (n_params + n_outs)
    out_specs = (PartitionSpec("core"),) * n_outs
    sharded = jax.jit(
        shard_map(_body, mesh=mesh, in_specs=in_specs, out_specs=out_specs,
                  check_rep=False),
        donate_argnums=donate, keep_unused=True)

    shardings = [NamedSharding(mesh, PartitionSpec("core"))] * n_outs
    zero_shapes = [(NCORES * a.shape[0],) + tuple(a.shape[1:])
                   for a in out_avals]
    zero_dtypes = [a.dtype for a in out_avals]

    def _zeros():
        return tuple(jnp.zeros(s, d) for s, d in zip(zero_shapes, zero_dtypes))

    zeros_fn = jax.jit(_zeros, out_shardings=tuple(shardings))
    runner = {"nc": nc, "sharded": sharded, "zeros_fn": zeros_fn,
              "in_names": in_names, "out_names": out_names,
              "out_avals": out_avals}
    _cache["runner"] = runner
    return runner


def _run(in_maps):
    import jax
    r = _get_runner()
    per_core = [[m[n] for n in r["in_names"]] for m in in_maps]
    concat_in = [np.concatenate([per_core[c][i] for c in range(NCORES)], axis=0)
                 for i in range(len(r["in_names"]))]
    zeros = r["zeros_fn"]()
    out_arrs = r["sharded"](*concat_in, *zeros)
    jax.block_until_ready(out_arrs)
    return out_arrs


def kernel(x, gn_w, gn_b, q_w, q_b, k_w, k_b, v_w, v_b, proj_w):
    import ml_dtypes

    x = np.asarray(x, np.float32)
    gn_w = np.asarray(gn_w, np.float32); gn_b = np.asarray(gn_b, np.float32)
    q_w = np.asarray(q_w, np.float32); q_b = np.asarray(q_b, np.float32)
    k_w = np.asarray(k_w, np.float32); k_b = np.asarray(k_b, np.float32)
    v_w = np.asarray(v_w, np.float32); v_b = np.asarray(v_b, np.float32)
    proj_w = np.asarray(proj_w, np.float32)

    # fold GroupNorm affine into conv weights (host, tiny)
    wq = np.ascontiguousarray((q_w * gn_w[None, :]).T, np.float32)
    wk = np.ascontiguousarray((k_w * gn_w[None, :]).T, np.float32)
    wv = np.ascontiguousarray((v_w * gn_w[None, :]).T, np.float32)
    bq = (q_b + q_w @ gn_b)
    bk = (k_b + k_w @ gn_b)
    bv = (v_b + v_w @ gn_b)
    wv_ext = np.ascontiguousarray(
        np.concatenate([wv, bv[None, :]], axis=0), np.float32)
    bqk = np.ascontiguousarray(np.stack([bq, bk, bv], axis=1), np.float32)
    pw = np.ascontiguousarray(proj_w.T, np.float32)

    x_bf = x.astype(ml_dtypes.bfloat16)
    in_maps = []
    for core in range(NCORES):
        bi, qi = core // 4, core % 4
        xs = np.ascontiguousarray(
            x_bf[bi, :, qi * QROWS:(qi + 1) * QROWS, :].reshape(C, Q))
        in_maps.append({"xq": xs, "wq_t": wq, "wk_t": wk, "wv_ext": wv_ext,
                        "pw_t": pw, "bqk": bqk})

    out_arrs = _run(in_maps)
    r = _cache["runner"]
    oq = np.asarray(out_arrs[0]).astype(np.float32)   # [8*64, 50176]
    oq = oq.reshape(NCORES, C, QROWS, SIZE)
    out = np.empty_like(x)
    for core in range(NCORES):
        bi, qi = core // 4, core % 4
        out[bi, :, qi * QROWS:(qi + 1) * QROWS, :] = oq[core]
    return out
